# revision 1
# baseline (speedup 1.0000x reference)
"""DistMult edge scoring on 8 Trainium2 NeuronCores.

score[e] = sum_d node_emb[src[e], d] * rel_emb[e, d] * node_emb[dst[e], d]

Strategy (data-parallel over edges, node table replicated per core):
  - Each of the 8 cores gets the full node_emb table in its HBM plus a
    1/8 shard of the edges (rel rows + src/dst indices).
  - The node table is split into 4 blocks of 25000 rows. Each core's
    edges are bucketed host-side into 16 groups by (src_block,
    dst_block), so within a group both gathers address a <32768-row
    window of the table and the fast GPSIMD dma_gather ucode (int16
    local indices, ~0.35ns/row descriptor generation) can be used
    instead of generic indirect DMA (~8ns/row).
  - Groups are padded to a fixed capacity (input-independent kernel
    shape) and processed in chunks of CH edges: two dma_gathers (head,
    tail) + one strided rel load land [128, CH/128, 128] f32 tiles with
    edge i of the chunk at [i%128, i//128, :]; DVE does two elementwise
    multiplies + a blocked reduce over D=128 into a resident score
    plane, stored to HBM once at the end.
  - Host pre-permutes rel rows into the chunk layout and inverts the
    edge permutation on the returned score planes.
"""

import numpy as np

N_NODES = 100000
E_TOTAL = 600000
D = 128
N_CORES = 8
E_CORE = E_TOTAL // N_CORES  # 75000

NB = 4                # node blocks
BS = N_NODES // NB    # block size (rows per gather window)
G = NB * NB           # groups per core
CH = 2560             # edge slots per chunk (20 cols of 128)
CAP = 5120            # slots per group (must be multiple of CH)
S = G * CAP           # total slots per core
COLS = S // 128       # score plane columns

_CACHE: dict = {}


def _build_module(repeats: int = 1):
    import concourse.bacc as bacc
    import concourse.mybir as mybir
    from concourse.tile import TileContext

    nc = bacc.Bacc(
        "TRN2",
        debug=False,
        enable_asserts=False,
        target_bir_lowering=False,
        num_devices=N_CORES,
    )
    f32 = mybir.dt.float32
    i16 = mybir.dt.int16

    node = nc.dram_tensor("node_emb", [N_NODES, D], f32, kind="ExternalInput").ap()
    relsw = nc.dram_tensor("relsw", [128, S], f32, kind="ExternalInput").ap()
    srci = nc.dram_tensor("srci", [128, S // 16], i16, kind="ExternalInput").ap()
    dsti = nc.dram_tensor("dsti", [128, S // 16], i16, kind="ExternalInput").ap()
    out = nc.dram_tensor("scores", [128, COLS], f32, kind="ExternalOutput").ap()

    n_chunks = CAP // CH

    with TileContext(nc) as tc:
        with (
            tc.tile_pool(name="idx", bufs=1) as idxp,
            tc.tile_pool(name="big", bufs=5) as bigp,
            tc.tile_pool(name="res", bufs=1) as resp,
        ):
            src_t = idxp.tile([128, S // 16], i16, tag="srci")
            dst_t = idxp.tile([128, S // 16], i16, tag="dsti")
            score_t = resp.tile([128, COLS], f32, tag="score")
            nc.sync.dma_start(out=src_t[:], in_=srci[:])
            nc.sync.dma_start(out=dst_t[:], in_=dsti[:])

            for _rep in range(repeats):
              for g in range(G):
                sb = (g // NB) * BS
                db = (g % NB) * BS
                for c in range(n_chunks):
                    s0 = g * CAP + c * CH
                    head = bigp.tile([128, CH], f32, tag="head")
                    tail = bigp.tile([128, CH], f32, tag="tail")
                    relt = bigp.tile([128, CH], f32, tag="rel")
                    nc.gpsimd.dma_gather(
                        out_ap=head[:].rearrange("p (c d) -> p c d", d=D),
                        in_ap=node[sb : sb + BS],
                        idxs_ap=src_t[:, s0 // 16 : (s0 + CH) // 16],
                        num_idxs=CH,
                        num_idxs_reg=CH,
                        elem_size=D,
                        single_packet=False,
                    )
                    nc.gpsimd.dma_gather(
                        out_ap=tail[:].rearrange("p (c d) -> p c d", d=D),
                        in_ap=node[db : db + BS],
                        idxs_ap=dst_t[:, s0 // 16 : (s0 + CH) // 16],
                        num_idxs=CH,
                        num_idxs_reg=CH,
                        elem_size=D,
                        single_packet=False,
                    )
                    nc.sync.dma_start(out=relt[:], in_=relsw[:, s0 : s0 + CH])
                    nc.vector.tensor_tensor(
                        out=head[:], in0=head[:], in1=relt[:],
                        op=mybir.AluOpType.mult,
                    )
                    nc.vector.tensor_tensor(
                        out=head[:], in0=head[:], in1=tail[:],
                        op=mybir.AluOpType.mult,
                    )
                    nc.vector.tensor_reduce(
                        out=score_t[:, s0 // 128 : (s0 + CH) // 128],
                        in_=head[:].rearrange("p (c d) -> p c d", d=D),
                        axis=mybir.AxisListType.X,
                        op=mybir.AluOpType.add,
                    )

            nc.sync.dma_start(out=out[:], in_=score_t[:])

    nc.compile()
    return nc


def _get_module(repeats: int = 1):
    key = ("nc", repeats)
    if key not in _CACHE:
        _CACHE[key] = _build_module(repeats)
    return _CACHE[key]


def _wrap16(x: np.ndarray) -> np.ndarray:
    """[S] int16 -> [128, S/16] gather index plane (16-wrap, replicated 8x)."""
    w = x.reshape(S // 16, 16).T
    return np.ascontiguousarray(np.tile(w, (8, 1)))


def _prep_core(rel_c, src_c, dst_c):
    src_c = src_c.astype(np.int64)
    dst_c = dst_c.astype(np.int64)
    g = (src_c // BS) * NB + (dst_c // BS)
    order = np.argsort(g, kind="stable")
    gs = g[order]
    counts = np.bincount(g, minlength=G)
    if counts.max() > CAP:
        raise ValueError(f"group overflow: {counts.max()} > CAP={CAP}")
    cum = np.zeros(G, dtype=np.int64)
    cum[1:] = np.cumsum(counts)[:-1]
    rank = np.arange(E_CORE) - cum[gs]
    slots = gs * CAP + rank  # slot for each sorted edge

    loc_src = np.zeros(S, dtype=np.int16)
    loc_dst = np.zeros(S, dtype=np.int16)
    loc_src[slots] = (src_c[order] - (gs // NB) * BS).astype(np.int16)
    loc_dst[slots] = (dst_c[order] - (gs % NB) * BS).astype(np.int16)

    rel_perm = np.zeros((S, D), dtype=np.float32)
    rel_perm[slots] = rel_c[order]
    relsw = np.ascontiguousarray(
        rel_perm.reshape(S // 128, 128, D).transpose(1, 0, 2).reshape(128, S)
    )
    return (
        {"relsw": relsw, "srci": _wrap16(loc_src), "dsti": _wrap16(loc_dst)},
        order,
        slots,
    )


def make_in_maps(node_emb, rel_emb, src, dst):
    node = np.ascontiguousarray(np.asarray(node_emb, dtype=np.float32))
    rel_emb = np.asarray(rel_emb, dtype=np.float32)
    src = np.asarray(src)
    dst = np.asarray(dst)
    in_maps, metas = [], []
    for c in range(N_CORES):
        sl = slice(c * E_CORE, (c + 1) * E_CORE)
        m, order, slots = _prep_core(rel_emb[sl], src[sl], dst[sl])
        m["node_emb"] = node
        in_maps.append(m)
        metas.append((order, slots))
    return in_maps, metas


def gather_outputs(results, metas) -> np.ndarray:
    scores = np.empty(E_TOTAL, dtype=np.float32)
    for c in range(N_CORES):
        plane = np.asarray(results[c]["scores"], dtype=np.float32)  # [128, COLS]
        lin = plane.T.ravel()  # lin[slot], slot = col*128 + p
        order, slots = metas[c]
        out_c = np.empty(E_CORE, dtype=np.float32)
        out_c[order] = lin[slots]
        scores[c * E_CORE : (c + 1) * E_CORE] = out_c
    return scores


def run(node_emb, rel_emb, src, dst, trace=False):
    from concourse import bass_utils
    from concourse.bass_interp import get_hw_module

    nc = _get_module()
    in_maps, metas = make_in_maps(node_emb, rel_emb, src, dst)
    old_m = nc.m
    nc.m = get_hw_module(nc.m)
    try:
        res = bass_utils.run_bass_kernel_spmd(
            nc, in_maps, core_ids=list(range(N_CORES)), trace=trace
        )
    finally:
        nc.m = old_m
    return gather_outputs(res.results, metas), res


def kernel(node_emb, rel_emb, src, dst):
    scores, _ = run(node_emb, rel_emb, src, dst, trace=False)
    return scores



# revision 2
# speedup vs baseline: 1.3907x; 1.3907x over previous
"""DistMult edge scoring on 8 Trainium2 NeuronCores.

score[e] = sum_d node_emb[src[e], d] * rel_emb[e, d] * node_emb[dst[e], d]

Strategy (data-parallel over edges, node table replicated per core):
  - Each core gets the full node table (bf16) in its HBM plus a 1/8 shard
    of the edges (rel rows in bf16 + src/dst indices).
  - The table is split into 4 blocks of 25000 rows; each core's edges are
    bucketed host-side into 16 groups by (src_block, dst_block) so both
    gathers of a group address a <32768-row window (int16 local indices).
  - KEY perf fix vs the original version: dma_gather descriptor generation
    runs on ONE Q7 core-pair selected by queue_num (ucode:
    `cpu_id / 2 == queue_num`; ~5-9 ns per gathered row).  With the default
    single SWDGE queue all gathers serialized on GPSIMD cores 0-1
    (~1.2 ms/core, the old bottleneck).  Building with num_swdge_queues=4
    and rotating queue_num 0-3 across gathers engages all four core-pairs:
    measured 7.4x faster for the same gather volume.
  - Gathers are per-group (one head + one tail dma_gather of ~4900 rows)
    with num_idxs statically trimmed to the group's true edge count
    (pad slots are never gathered; their rel rows are zero, which zeroes
    any stale data the compute reads there).
  - Everything is bf16: halves HBM gather/stream bytes and doubles DVE
    elementwise rate (2X_1PORT, which never contends with the GPSIMD
    descriptor work).  The D=128 reduce is a log-reduction: two bf16
    2x-rate tensor_tensor adds (128->64->32) + one 1x tensor_reduce over
    32, accumulating into an fp32 score plane (tensor_reduce has no 2x
    micro-op, so pre-halving on the 2x path is cheaper).
  - l2 relative error ~4.4e-3 (gate 2e-2), dominated by bf16 rounding.
"""

import numpy as np
import ml_dtypes

N_NODES = 100000
E_TOTAL = 600000
D = 128
N_CORES = 8
E_CORE = E_TOTAL // N_CORES  # 75000

NB = 4                # node blocks
BS = N_NODES // NB    # block size (rows per gather window)
G = NB * NB           # groups per core
CAP = 4864            # slots per group (auto-raised on overflow; mult of 256)
NQ = 4                # SWDGE queues (4 Q7 core-pairs generate descriptors)


def _geom(cap):
    ch = cap // 2          # edge slots per chunk
    s = G * cap            # total slots per core
    return ch, s, s // 128


CH, S, COLS = _geom(CAP)
GROUP_NIDX = tuple([CAP] * G)  # per-group gather lengths, set by make_in_maps


def _set_cap(cap):
    """Adopt a group capacity (multiple of 256) and update the geometry."""
    global CAP, CH, S, COLS
    CAP = cap
    CH, S, COLS = _geom(cap)

_CACHE: dict = {}


def _build_module(repeats: int = 1, cap: int | None = None):
    cap = CAP if cap is None else cap
    CAP = cap
    CH, S, COLS = _geom(cap)
    import concourse.bacc as bacc
    import concourse.mybir as mybir
    from concourse.tile import TileContext

    nc = bacc.Bacc(
        "TRN2",
        debug=False,
        enable_asserts=False,
        target_bir_lowering=False,
        num_devices=N_CORES,
        num_swdge_queues=NQ,
    )
    f32 = mybir.dt.float32
    bf16 = mybir.dt.bfloat16
    i16 = mybir.dt.int16

    nodeb = nc.dram_tensor("nodeb", [N_NODES, D], bf16, kind="ExternalInput").ap()
    relsw = nc.dram_tensor("relsw", [128, S], bf16, kind="ExternalInput").ap()
    srci = nc.dram_tensor("srci", [128, S // 16], i16, kind="ExternalInput").ap()
    dsti = nc.dram_tensor("dsti", [128, S // 16], i16, kind="ExternalInput").ap()
    out = nc.dram_tensor("scores", [128, COLS], f32, kind="ExternalOutput").ap()

    n_chunks = CAP // CH

    group_nidx = GROUP_NIDX
    with TileContext(nc) as tc:
        with (
            tc.tile_pool(name="idx", bufs=1) as idxp,
            tc.tile_pool(name="big", bufs=6) as bigp,
            tc.tile_pool(name="gat", bufs=3) as gatp,
            tc.tile_pool(name="res", bufs=1) as resp,
        ):
            src_t = idxp.tile([128, S // 16], i16, tag="srci")
            dst_t = idxp.tile([128, S // 16], i16, tag="dsti")
            score_t = resp.tile([128, COLS], f32, tag="score")
            nc.sync.dma_start(out=src_t[:], in_=srci[:])
            nc.sync.dma_start(out=dst_t[:], in_=dsti[:])

            qctr = 0
            for _rep in range(repeats):
              for g in range(G):
                sb = (g // NB) * BS
                db = (g % NB) * BS
                g0 = g * CAP
                nid = group_nidx[g]
                ncols = -(-nid // 128)
                headC = gatp.tile([128, CAP], bf16, tag="headC")
                tailC = gatp.tile([128, CAP], bf16, tag="tailC")
                for tile, idxt, base in ((headC, src_t, sb), (tailC, dst_t, db)):
                    nc.gpsimd.dma_gather(
                        out_ap=tile[:, : ncols * D].rearrange(
                            "p (c d) -> p c d", d=D
                        ),
                        in_ap=nodeb[base : base + BS],
                        idxs_ap=idxt[:, g0 // 16 : g0 // 16 + nid // 16],
                        num_idxs=nid,
                        num_idxs_reg=nid,
                        elem_size=D,
                        single_packet=False,
                        queue_num=qctr % NQ,
                    )
                    qctr += 1
                for c in range(n_chunks):
                    s0 = g0 + c * CH
                    head = headC[:, c * CH : (c + 1) * CH]
                    tail = tailC[:, c * CH : (c + 1) * CH]
                    relt = bigp.tile([128, CH], bf16, tag="rel")
                    nc.sync.dma_start(out=relt[:], in_=relsw[:, s0 : s0 + CH])
                    nc.vector.tensor_tensor(
                        out=head, in0=head, in1=relt[:],
                        op=mybir.AluOpType.mult,
                    )
                    nc.vector.tensor_tensor(
                        out=head, in0=head, in1=tail,
                        op=mybir.AluOpType.mult,
                    )
                    # log-reduce over D: two bf16 2x-rate adds halve 128->32,
                    # then a single 1x tensor_reduce over the remaining 32.
                    h3 = head.rearrange("p (c d) -> p c d", d=D)
                    nc.vector.tensor_tensor(
                        out=h3[:, :, 0:64], in0=h3[:, :, 0:64], in1=h3[:, :, 64:128],
                        op=mybir.AluOpType.add,
                    )
                    nc.vector.tensor_tensor(
                        out=h3[:, :, 0:32], in0=h3[:, :, 0:32], in1=h3[:, :, 32:64],
                        op=mybir.AluOpType.add,
                    )
                    nc.vector.tensor_reduce(
                        out=score_t[:, s0 // 128 : (s0 + CH) // 128],
                        in_=h3[:, :, 0:32],
                        axis=mybir.AxisListType.X,
                        op=mybir.AluOpType.add,
                    )

            nc.sync.dma_start(out=out[:], in_=score_t[:])

    nc.compile()
    return nc


def _get_module(repeats: int = 1, cap: int | None = None):
    cap = CAP if cap is None else cap
    key = ("nc", repeats, cap, GROUP_NIDX)
    if key not in _CACHE:
        _CACHE[key] = _build_module(repeats, cap)
    return _CACHE[key]


def _wrap16(x: np.ndarray) -> np.ndarray:
    """[S] int16 -> [128, S/16] gather index plane (16-wrap, replicated 8x)."""
    w = x.reshape(S // 16, 16).T
    return np.ascontiguousarray(np.tile(w, (8, 1)))


def _prep_core(rel_c, src_c, dst_c):
    src_c = src_c.astype(np.int64)
    dst_c = dst_c.astype(np.int64)
    g = (src_c // BS) * NB + (dst_c // BS)
    order = np.argsort(g, kind="stable")
    gs = g[order]
    counts = np.bincount(g, minlength=G)
    if counts.max() > CAP:
        raise ValueError(f"group overflow: {counts.max()} > CAP={CAP}")
    cum = np.zeros(G, dtype=np.int64)
    cum[1:] = np.cumsum(counts)[:-1]
    rank = np.arange(E_CORE) - cum[gs]
    slots = gs * CAP + rank  # slot for each sorted edge

    loc_src = np.zeros(S, dtype=np.int16)
    loc_dst = np.zeros(S, dtype=np.int16)
    loc_src[slots] = (src_c[order] - (gs // NB) * BS).astype(np.int16)
    loc_dst[slots] = (dst_c[order] - (gs % NB) * BS).astype(np.int16)

    rel_perm = np.zeros((S, D), dtype=np.float32)
    rel_perm[slots] = rel_c[order]
    relsw = np.ascontiguousarray(
        rel_perm.reshape(S // 128, 128, D)
        .transpose(1, 0, 2)
        .reshape(128, S)
        .astype(ml_dtypes.bfloat16)
    )
    return (
        {"relsw": relsw, "srci": _wrap16(loc_src), "dsti": _wrap16(loc_dst)},
        order,
        slots,
    )


def make_in_maps(node_emb, rel_emb, src, dst):
    node = np.asarray(node_emb, dtype=np.float32)
    nodeb = np.ascontiguousarray(node.astype(ml_dtypes.bfloat16))
    rel_emb = np.asarray(rel_emb, dtype=np.float32)
    src = np.asarray(src)
    dst = np.asarray(dst)

    # Size group capacity to the data (deterministic inputs -> 4864), with
    # headroom raises if a different input distribution overflows.
    mx = 0
    for c in range(N_CORES):
        sl = slice(c * E_CORE, (c + 1) * E_CORE)
        g = (src[sl].astype(np.int64) // BS) * NB + dst[sl].astype(np.int64) // BS
        mx = max(mx, int(np.bincount(g, minlength=G).max()))
    cap = max(4864, -(-mx // 256) * 256)
    if cap != CAP:
        _set_cap(cap)

    # Per-group gather lengths (max across cores, rounded up to 16): the
    # gathers skip pad slots entirely; their rel rows are zero so stale
    # gather-tile columns contribute nothing.
    global GROUP_NIDX
    gmax = np.zeros(G, dtype=np.int64)
    for c in range(N_CORES):
        sl = slice(c * E_CORE, (c + 1) * E_CORE)
        g = (src[sl].astype(np.int64) // BS) * NB + dst[sl].astype(np.int64) // BS
        gmax = np.maximum(gmax, np.bincount(g, minlength=G))
    GROUP_NIDX = tuple(int(-(-int(x) // 16) * 16) for x in gmax)

    in_maps, metas = [], []
    for c in range(N_CORES):
        sl = slice(c * E_CORE, (c + 1) * E_CORE)
        m, order, slots = _prep_core(rel_emb[sl], src[sl], dst[sl])
        m["nodeb"] = nodeb
        in_maps.append(m)
        metas.append((order, slots))
    return in_maps, metas


def gather_outputs(results, metas) -> np.ndarray:
    scores = np.empty(E_TOTAL, dtype=np.float32)
    for c in range(N_CORES):
        plane = np.asarray(results[c]["scores"], dtype=np.float32)  # [128, COLS]
        lin = plane.T.ravel()  # lin[slot], slot = col*128 + p
        order, slots = metas[c]
        out_c = np.empty(E_CORE, dtype=np.float32)
        out_c[order] = lin[slots]
        scores[c * E_CORE : (c + 1) * E_CORE] = out_c
    return scores


def run(node_emb, rel_emb, src, dst, trace=False):
    from concourse import bass_utils
    from concourse.bass_interp import get_hw_module

    in_maps, metas = make_in_maps(node_emb, rel_emb, src, dst)
    nc = _get_module()
    old_m = nc.m
    nc.m = get_hw_module(nc.m)
    try:
        res = bass_utils.run_bass_kernel_spmd(
            nc, in_maps, core_ids=list(range(N_CORES)), trace=trace
        )
    finally:
        nc.m = old_m
    return gather_outputs(res.results, metas), res


def kernel(node_emb, rel_emb, src, dst):
    scores, _ = run(node_emb, rel_emb, src, dst, trace=False)
    return scores
